# revision 11
# baseline (speedup 1.0000x reference)
"""Fastformer (additive attention) Bass kernel for Trainium2, 8-core data-parallel.

Math (per batch element b, algebraic collapse of the reference):
    A_q   = Wq @ Wqa                                    [768, 12]  (host weight prep)
    s_q   = x @ A_q ;  e_q = exp(s_q/8 + lm/8)          [S, 12]
    xw_q  = e_q^T @ x ; den_q = sum_s e_q               [12,768], [12]
    q_ctx = diag-blocks of ((xw_q/den_q) @ Wq)          [768]
    A_k   = Wk @ (q_ctx * Wka); same pooling -> kc0     [768]
    k_ctx = q_ctx * kc0
    M     = Wq @ (blockdiag_h(k_ctx_h * Wo) + I)        [768, 768]
    out   = x @ M                                       [S, 768]

Pooling-path matmuls are oriented so outputs have tiny free dims; the big
x @ M pass runs as a 3-term error-compensated fp8(e4m3) DoubleRow matmul:
    out = x8@M8 + x8@Mr8 + xr8@M8     (xr = x - x8, Mr = M - M8, PSUM x64)
Sharding: batch b -> core b (B == n_cores == 8).
"""
import math
from contextlib import ExitStack

import numpy as np
import ml_dtypes

import concourse.bass as bass
import concourse.bacc as bacc
import concourse.tile as tile
import concourse.mybir as mybir

F8 = mybir.dt.float8e4
F16 = mybir.dt.float16
F32 = mybir.dt.float32
NP8 = ml_dtypes.float8_e4m3

B, S, F, H, D = 8, 4096, 768, 12, 64
P = 128
NF = F // P            # 6 feature chunks
NS = S // P            # 32 seq chunks
GS = 4                 # seq chunks per score group
NG = NS // GS          # 8 groups
N_CORES = 8
ESC = 1.0 / math.sqrt(D)   # exp scale 1/8
MS = 64.0                  # M-side PSUM scale (power of two)
DR = mybir.MatmulPerfMode.DoubleRow

_prog_cache = {}


def _emit_scores(nc, pools, cst, A3, masked, half, e_tiles):
    """Scores + exp for groups covered by x8t column half `half`."""
    psS, ework = pools["psS"], pools["ework"]
    x8t3, lm_sb = cst["x8t3"], cst["lm_sb"]
    for g in range(NG // 2 * half, NG // 2 * (half + 1)):
        sc = psS.tile([P, GS * 12], F32, tag="sc")
        for r in range(GS):
            i = GS * g + r
            for j in range(NF):
                nc.tensor.matmul(sc[:, 12 * r:12 * (r + 1)],
                                 x8t3[:, j, P * i:P * (i + 1)], A3[:, j, :],
                                 start=(j == 0), stop=(j == NF - 1))
        e8 = ework.tile([P, GS * 12], F8, tag=f"e{g}")
        if masked:
            for r in range(GS):
                i = GS * g + r
                nc.scalar.activation(e8[:, 12 * r:12 * (r + 1)],
                                     sc[:, 12 * r:12 * (r + 1)],
                                     mybir.ActivationFunctionType.Exp,
                                     bias=lm_sb[:, i:i + 1], scale=ESC)
        else:
            nc.scalar.activation(e8[:], sc[:],
                                 mybir.ActivationFunctionType.Exp, scale=ESC)
        e_tiles.append(e8)


def _emit_xw(nc, pools, cst, e_tiles, groups, xw, first_i, last_i):
    """Accumulate xw/den over the given groups into xw (PSUM [P,(NF+1)*12])."""
    ones128_8, xs8 = cst["ones128_8"], cst["xs8"]
    xw3 = xw[:].rearrange("p (a b) -> p a b", a=NF + 1)
    for g in groups:
        e8 = e_tiles[g]
        for r in range(GS):
            i = GS * g + r
            first, last = (i == first_i), (i == last_i)
            rhs = e8[:, 12 * r:12 * (r + 1)]
            for j in range(NF):
                nc.tensor.matmul(xw3[:, j, :],
                                 xs8[g][:, r, P * j:P * (j + 1)], rhs,
                                 start=first, stop=last)
            nc.tensor.matmul(xw3[:, NF, :], ones128_8[:], rhs,
                             start=first, stop=last)


def _emit_ctx(nc, pools, xw, W3, tag):
    """xw/den -> xq8 -> diagonal-head G entries -> ctx [128, NF] f32 (SBUF)."""
    psG, ework = pools["psG"], pools["ework"]
    xw3 = xw[:].rearrange("p (a b) -> p a b", a=NF + 1)

    # den column holds 64*den (ones tile = 64), so inv = (1/64)/den and the
    # x64-scaled W chunks (wq8s/wk8s) cancel it in GT.
    inv = ework.tile([P, 12], F32, tag=f"inv{tag}")
    nc.vector.reciprocal(inv[:], xw[:, NF * 12:NF * 12 + 12])

    xq8 = ework.tile([P, NF * 12], F8, tag=f"xq{tag}")
    xq3 = xq8[:].rearrange("p (a b) -> p a b", a=NF)
    nc.vector.tensor_tensor(xq3, xw3[:, 0:NF, :],
                            inv[:, None, :].broadcast_to((P, NF, 12)),
                            mybir.AluOpType.mult)

    # only diagonal head pairs of G are needed: block m uses heads 2m, 2m+1
    gt = psG.tile([P, 2 * NF], F32, tag="g")
    for m in range(NF):
        for j in range(NF):
            nc.tensor.matmul(gt[:, 2 * m:2 * (m + 1)],
                             W3[:, j, P * m:P * (m + 1)],
                             xq3[:, j, 2 * m:2 * (m + 1)],
                             start=(j == 0), stop=(j == NF - 1))
    gt3 = gt[:].rearrange("p (a b) -> p a b", a=NF)
    return gt3


def build_program(masked=False):
    nc = bacc.Bacc(trn_type="TRN2", target_bir_lowering=False)

    x8t_d = nc.dram_tensor("x8t", [P, NF * S], F8, kind="ExternalInput")
    xr8t_d = nc.dram_tensor("xr8t", [P, NF * S], F8, kind="ExternalInput")
    xs8_d = nc.dram_tensor("xs8", [P, NS * F], F8, kind="ExternalInput")
    aq8_d = nc.dram_tensor("aq8", [P, NF * 12], F8, kind="ExternalInput")
    wq8_d = nc.dram_tensor("wq8", [P, NF * F], F8, kind="ExternalInput")
    wk8_d = nc.dram_tensor("wk8", [P, NF * F], F8, kind="ExternalInput")
    wkt8_d = nc.dram_tensor("wkt8", [P, NF * F], F8, kind="ExternalInput")
    wqt8_d = nc.dram_tensor("wqt8", [P, NF * F], F8, kind="ExternalInput")
    wqr8_d = nc.dram_tensor("wqr8", [P, NF * F], F8, kind="ExternalInput")
    wka_d = nc.dram_tensor("wka", [P, NF * 12], F32, kind="ExternalInput")
    wobd_d = nc.dram_tensor("wobd", [P, P], F32, kind="ExternalInput")
    ones8_d = nc.dram_tensor("ones8", [P, P], F8, kind="ExternalInput")
    lm_d = nc.dram_tensor("lm", [P, NS], F32, kind="ExternalInput")
    out_d = nc.dram_tensor("out", [S, F], F16, kind="ExternalOutput")

    with tile.TileContext(nc) as tc:
        with ExitStack() as ctx:
            cpool = ctx.enter_context(tc.tile_pool(name="const", bufs=1))
            ework = ctx.enter_context(tc.tile_pool(name="ework", bufs=1))
            obuf = ctx.enter_context(tc.tile_pool(name="obuf", bufs=3))
            psW = ctx.enter_context(tc.tile_pool(name="psW", bufs=2, space="PSUM"))

            # ---- loads, in consumption order; small tensors first
            aq8 = cpool.tile([P, NF * 12], F8, tag="aq8")
            nc.sync.dma_start(aq8[:], aq8_d[:])
            ones128_8 = cpool.tile([P, P], F8, tag="ones8")
            nc.sync.dma_start(ones128_8[:], ones8_d[:])
            lm_sb = cpool.tile([P, NS], F32, tag="lm")
            if masked:
                nc.sync.dma_start(lm_sb[:], lm_d[:])
            x8t = cpool.tile([P, NF * S], F8, tag="x8t")
            x8t3 = x8t[:].rearrange("p (a b) -> p a b", a=NF)
            x8t_d3 = x8t_d[:].rearrange("p (a b) -> p a b", a=NF)
            xs8_tiles = []
            xs8 = []
            for g in range(NG):
                t = cpool.tile([P, GS * F], F8, tag=f"xs8_{g}")
                xs8_tiles.append(t)
                xs8.append(t[:].rearrange("p (a b) -> p a b", a=GS))
            # interleave: xT half 1, xs groups 0-3, xT half 2, xs groups 4-7
            nc.sync.dma_start(x8t3[:, :, 0:S // 2], x8t_d3[:, :, 0:S // 2])
            for g in range(NG // 2):
                nc.sync.dma_start(xs8_tiles[g][:],
                                  xs8_d[:, GS * F * g:GS * F * (g + 1)])
            nc.sync.dma_start(x8t3[:, :, S // 2:S], x8t_d3[:, :, S // 2:S])
            for g in range(NG // 2, NG):
                nc.sync.dma_start(xs8_tiles[g][:],
                                  xs8_d[:, GS * F * g:GS * F * (g + 1)])

            wq8 = cpool.tile([P, NF * F], F8, tag="wq8")
            nc.sync.dma_start(wq8[:], wq8_d[:])
            wkt8 = cpool.tile([P, NF * F], F8, tag="wkt8")
            nc.sync.dma_start(wkt8[:], wkt8_d[:])
            wka = cpool.tile([P, NF * 12], F32, tag="wka")
            nc.sync.dma_start(wka[:], wka_d[:])
            wobd = cpool.tile([P, P], F32, tag="wobd")
            nc.sync.dma_start(wobd[:], wobd_d[:])
            wk8 = cpool.tile([P, NF * F], F8, tag="wk8")
            nc.sync.dma_start(wk8[:], wk8_d[:])
            wqt8 = cpool.tile([P, NF * F], F8, tag="wqt8")
            nc.sync.dma_start(wqt8[:], wqt8_d[:])
            wqr8 = cpool.tile([P, NF * F], F8, tag="wqr8")
            nc.sync.dma_start(wqr8[:], wqr8_d[:])
            xr8t = cpool.tile([P, NF * S], F8, tag="xr8t")
            xr8t3 = xr8t[:].rearrange("p (a b) -> p a b", a=NF)
            xr8t_d3 = xr8t_d[:].rearrange("p (a b) -> p a b", a=NF)
            for q in range(4):
                lo, hi = S // 4 * q, S // 4 * (q + 1)
                nc.sync.dma_start(xr8t3[:, :, lo:hi], xr8t_d3[:, :, lo:hi])

            wq3 = wq8[:].rearrange("p (a b) -> p a b", a=NF)
            wk3 = wk8[:].rearrange("p (a b) -> p a b", a=NF)
            wkt3 = wkt8[:].rearrange("p (a b) -> p a b", a=NF)
            wqt3 = wqt8[:].rearrange("p (a b) -> p a b", a=NF)
            wqr3 = wqr8[:].rearrange("p (a b) -> p a b", a=NF)
            wka3 = wka[:].rearrange("p (a b) -> p a b", a=NF)
            aq3 = aq8[:].rearrange("p (a b) -> p a b", a=NF)
            cst = {"x8t3": x8t3, "xs8": xs8, "ones128_8": ones128_8,
                   "lm_sb": lm_sb}

            mr8 = ework.tile([P, NF * F], F8, tag="mr8")
            m8_3 = wq3
            mr8_3 = mr8[:].rearrange("p (a b) -> p a b", a=NF)

            with ExitStack() as pre:
                psS = pre.enter_context(tc.tile_pool(name="psS", bufs=2,
                                                     space="PSUM"))
                psXW = pre.enter_context(tc.tile_pool(name="psXW", bufs=1,
                                                      space="PSUM"))
                psG = pre.enter_context(tc.tile_pool(name="psG", bufs=1,
                                                     space="PSUM"))
                pools = {"psS": psS, "psXW": psXW, "psG": psG, "ework": ework}

                # ---- pass 1: query pooling + q_ctx (split by xT halves)
                e_q = []
                xw_q = psXW.tile([P, (NF + 1) * 12], F32, tag="xw")
                _emit_scores(nc, pools, cst, aq3, masked, 0, e_q)
                _emit_xw(nc, pools, cst, e_q, range(NG // 2), xw_q, 0, NS - 1)
                _emit_scores(nc, pools, cst, aq3, masked, 1, e_q)
                _emit_xw(nc, pools, cst, e_q, range(NG // 2, NG), xw_q,
                         0, NS - 1)
                gtq3 = _emit_ctx(nc, pools, xw_q, wq3, "q")

                # ---- A_k = Wk @ (q_ctx * Wka): gate straight from PSUM gt
                g8 = ework.tile([P, NF * 12], F8, tag="g8")
                g3 = g8[:].rearrange("p (a b) -> p a b", a=NF)
                nc.vector.tensor_tensor(
                    g3[0:64], wka3[0:64],
                    gtq3[0:64, :, 0:1].broadcast_to((64, NF, 12)),
                    mybir.AluOpType.mult)
                nc.vector.tensor_tensor(
                    g3[64:P], wka3[64:P],
                    gtq3[64:P, :, 1:2].broadcast_to((64, NF, 12)),
                    mybir.AluOpType.mult)
                qctx = ework.tile([P, NF], F32, tag="qctx")
                nc.vector.tensor_copy(qctx[0:64, :], gtq3[0:64, :, 0])
                nc.vector.tensor_copy(qctx[64:P, :], gtq3[64:P, :, 1])
                ak_ps = psG.tile([P, NF * 12], F32, tag="g")
                for ft in range(NF):
                    for fc in range(NF):
                        nc.tensor.matmul(ak_ps[:, 12 * ft:12 * (ft + 1)],
                                         wkt3[:, fc, P * ft:P * (ft + 1)],
                                         g3[:, fc, :],
                                         start=(fc == 0), stop=(fc == NF - 1))
                ak8 = ework.tile([P, NF * 12], F8, tag="ak8")
                nc.scalar.copy(ak8[:], ak_ps[:])
                ak3 = ak8[:].rearrange("p (a b) -> p a b", a=NF)

                # ---- pass 2: key pooling + k_ctx
                e_k = []
                xw_k = psXW.tile([P, (NF + 1) * 12], F32, tag="xw")
                _emit_scores(nc, pools, cst, ak3, masked, 0, e_k)
                _emit_xw(nc, pools, cst, e_k, range(NG // 2), xw_k, 0, NS - 1)
                _emit_scores(nc, pools, cst, ak3, masked, 1, e_k)
                _emit_xw(nc, pools, cst, e_k, range(NG // 2, NG), xw_k,
                         0, NS - 1)
                gtk3 = _emit_ctx(nc, pools, xw_k, wk3, "k")
                kctx = ework.tile([P, NF], F32, tag="kctx")
                nc.vector.tensor_tensor(kctx[0:64, :], qctx[0:64, :],
                                        gtk3[0:64, :, 0],
                                        mybir.AluOpType.mult)
                nc.vector.tensor_tensor(kctx[64:P, :], qctx[64:P, :],
                                        gtk3[64:P, :, 1],
                                        mybir.AluOpType.mult)

                # ---- M = Wq @ (blockdiag(kctx_h * Wo) + I), scaled by MS
                # r_all[:, j, :] = wobd (block-diag stacked Wo, x64) row-scaled
                # by kctx[:, j]; the +I (i.e. + Wq) lands via MS*I128 matmuls.
                r_all = ework.tile([P, NF * P], F16, tag="r_all")
                r3 = r_all[:].rearrange("p (a b) -> p a b", a=NF)
                nc.vector.tensor_tensor(
                    r3, wobd[:, None, :].broadcast_to((P, NF, P)),
                    kctx[:, :, None].broadcast_to((P, NF, P)),
                    mybir.AluOpType.mult)

                for ft in range(NF):
                    mc = psW.tile([P, F], F32, tag="wide")
                    for j in range(NF):
                        nc.tensor.matmul(mc[:, P * j:P * (j + 1)],
                                         wqt3[:, j, P * ft:P * (ft + 1)],
                                         r3[:, j, :], start=True, stop=True)
                    nc.vector.tensor_tensor(mr8_3[:, ft, :], mc[:],
                                            wqr3[:, ft, :],
                                            mybir.AluOpType.add)

            # ---- pass 3: out = (x8 + xr8) @ M8 + x8 @ Mr8, fp8 DoubleRow,
            # pair-major accumulation; two pools alternate -> 4 chunks in flight
            psT = ctx.enter_context(tc.tile_pool(name="psT", bufs=2,
                                                 space="PSUM"))
            for i in range(NS):
                ps = (psW if i % 2 == 0 else psT).tile([P, F], F32, tag="wide")
                n = 0
                for t in range(NF // 2):
                    for lhs3, rhs3 in ((x8t3, m8_3), (x8t3, mr8_3),
                                      (xr8t3, m8_3)):
                        for lo, hi in ((0, 512), (512, F)):
                            nc.tensor.matmul(
                                ps[:, lo:hi],
                                lhs3[:, 2 * t:2 * t + 2, P * i:P * (i + 1)],
                                rhs3[:, 2 * t:2 * t + 2, lo:hi],
                                start=(n == 0), stop=(n == 16),
                                perf_mode=DR)
                        n += 2
                ow = obuf.tile([P, F], F16, tag="ow")
                nc.scalar.mul(ow[:], ps[:], 1.0 / MS)
                nc.sync.dma_start(out_d[P * i:P * (i + 1), :], ow[:])

    nc.compile()
    return nc


def _get_program(masked=False):
    key = ("m" if masked else "u")
    if key not in _prog_cache:
        _prog_cache[key] = build_program(masked)
    return _prog_cache[key]


def _chunk_rows(a, np_dtype):
    """[R*128, C] -> [128, R*C] with chunk r of rows at cols [r*C:(r+1)*C]."""
    R = a.shape[0] // P
    return np.ascontiguousarray(
        a.reshape(R, P, a.shape[1]).transpose(1, 0, 2).reshape(P, -1)
    ).astype(np_dtype)


def _prep_weights(Wq, Wk, Wqa, Wka, Wo):
    Aq = (Wq @ Wqa).astype(np.float32)
    wobd = np.zeros((P, P), np.float32)
    wobd[0:64, 0:64] = MS * Wo
    wobd[64:P, 64:P] = MS * Wo
    return {
        "aq8": _chunk_rows(Aq, NP8),
        "wq8": _chunk_rows(MS * Wq, NP8),
        "wk8": _chunk_rows(MS * Wk, NP8),
        "wkt8": _chunk_rows(np.ascontiguousarray(Wk.T), NP8),
        "wqt8": _chunk_rows(np.ascontiguousarray(Wq.T), NP8),
        "wqr8": _chunk_rows(
            MS * Wq - (MS * Wq).astype(NP8).astype(np.float32), NP8),
        "wka": _chunk_rows(Wka, np.float32),
        "wobd": wobd,
        "ones8": np.full((P, P), MS, NP8),
    }


def _prep_core_inputs(xb, maskb, w, masked):
    x8 = xb.astype(NP8)
    xr8 = (xb - x8.astype(np.float32)).astype(NP8)
    d = {
        "x8t": _chunk_rows(np.ascontiguousarray(x8.astype(np.float32).T), NP8),
        "xr8t": _chunk_rows(np.ascontiguousarray(xr8.astype(np.float32).T), NP8),
        "xs8": _chunk_rows(x8.astype(np.float32), NP8),
        "lm": np.zeros((P, NS), np.float32),
        **w,
    }
    if masked:
        lm = np.where(maskb > 0, 0.0, -60000.0).astype(np.float32) * ESC
        d["lm"] = np.ascontiguousarray(lm.reshape(NS, P).T)
    return d


def run(x, attn_mask, Wq, Wk, Wqa, Wka, Wo, trace=False):
    from concourse.bass_utils import run_bass_kernel_spmd

    masked = not bool(np.all(attn_mask == 1.0))
    nc = _get_program(masked)
    w = _prep_weights(Wq, Wk, Wqa, Wka, Wo)
    in_maps = [_prep_core_inputs(np.asarray(x[b]), np.asarray(attn_mask[b]),
                                 w, masked)
               for b in range(N_CORES)]
    res = run_bass_kernel_spmd(nc, in_maps, list(range(N_CORES)), trace=trace)
    out = np.stack([res.results[b]["out"].astype(np.float32)
                    for b in range(N_CORES)])
    return out, res


def kernel(x, attn_mask, Wq, Wk, Wqa, Wka, Wo):
    out, _ = run(np.asarray(x, dtype=np.float32),
                 np.asarray(attn_mask, dtype=np.float32),
                 np.asarray(Wq, dtype=np.float32),
                 np.asarray(Wk, dtype=np.float32),
                 np.asarray(Wqa, dtype=np.float32),
                 np.asarray(Wka, dtype=np.float32),
                 np.asarray(Wo, dtype=np.float32))
    return out


# revision 14
# speedup vs baseline: 1.0031x; 1.0031x over previous
"""Fastformer (additive attention) Bass kernel for Trainium2, 8-core data-parallel.

Math (per batch element b, algebraic collapse of the reference):
    A_q   = Wq @ Wqa                                    [768, 12]  (host weight prep)
    s_q   = x @ A_q ;  e_q = exp(s_q/8 + lm/8)          [S, 12]
    xw_q  = e_q^T @ x ; den_q = sum_s e_q               [12,768], [12]
    q_ctx = diag-blocks of ((xw_q/den_q) @ Wq)          [768]
    A_k   = Wk @ (q_ctx * Wka); same pooling -> kc0     [768]
    k_ctx = q_ctx * kc0
    M     = Wq @ (blockdiag_h(k_ctx_h * Wo) + I)        [768, 768]
    out   = x @ M                                       [S, 768]

Pooling-path matmuls are oriented so outputs have tiny free dims; the big
x @ M pass runs as a 3-term error-compensated fp8(e4m3) DoubleRow matmul:
    out = x8@M8 + x8@Mr8 + xr8@M8     (xr = x - x8, Mr = M - M8, PSUM x64)
Sharding: batch b -> core b (B == n_cores == 8).
"""
import math
from contextlib import ExitStack

import numpy as np
import ml_dtypes

import concourse.bass as bass
import concourse.bacc as bacc
import concourse.tile as tile
import concourse.mybir as mybir

F8 = mybir.dt.float8e4
F16 = mybir.dt.float16
F32 = mybir.dt.float32
NP8 = ml_dtypes.float8_e4m3

B, S, F, H, D = 8, 4096, 768, 12, 64
P = 128
NF = F // P            # 6 feature chunks
NS = S // P            # 32 seq chunks
GS = 4                 # seq chunks per score group
NG = NS // GS          # 8 groups
N_CORES = 8
ESC = 1.0 / math.sqrt(D)   # exp scale 1/8
MS = 64.0                  # M-side PSUM scale (power of two)
DR = mybir.MatmulPerfMode.DoubleRow

_prog_cache = {}


def _emit_scores(nc, pools, cst, A3, masked, half, e_tiles):
    """Scores + exp for groups covered by x8t column half `half`."""
    psS, ework = pools["psS"], pools["ework"]
    x8t3, lm_sb = cst["x8t3"], cst["lm_sb"]
    for g in range(NG // 2 * half, NG // 2 * (half + 1)):
        sc = psS.tile([P, GS * 12], F32, tag="sc")
        for r in range(GS):
            i = GS * g + r
            for j in range(NF):
                nc.tensor.matmul(sc[:, 12 * r:12 * (r + 1)],
                                 x8t3[:, j, P * i:P * (i + 1)], A3[:, j, :],
                                 start=(j == 0), stop=(j == NF - 1))
        e8 = ework.tile([P, GS * 12], F8, tag=f"e{g}")
        if masked:
            for r in range(GS):
                i = GS * g + r
                nc.scalar.activation(e8[:, 12 * r:12 * (r + 1)],
                                     sc[:, 12 * r:12 * (r + 1)],
                                     mybir.ActivationFunctionType.Exp,
                                     bias=lm_sb[:, i:i + 1], scale=ESC)
        else:
            nc.scalar.activation(e8[:], sc[:],
                                 mybir.ActivationFunctionType.Exp, scale=ESC)
        e_tiles.append(e8)


def _emit_xw(nc, pools, cst, e_tiles, groups, xw, first_i, last_i):
    """Accumulate xw/den over the given groups into xw (PSUM [P,(NF+1)*12])."""
    ones1_8, xs8 = cst["ones1_8"], cst["xs8"]
    xw3 = xw[:].rearrange("p (a b) -> p a b", a=NF + 1)
    for g in groups:
        e8 = e_tiles[g]
        for r in range(GS):
            i = GS * g + r
            first, last = (i == first_i), (i == last_i)
            rhs = e8[:, 12 * r:12 * (r + 1)]
            for j in range(NF):
                nc.tensor.matmul(xw3[:, j, :],
                                 xs8[g][:, r, P * j:P * (j + 1)], rhs,
                                 start=first, stop=last)
            nc.tensor.matmul(xw3[:, NF, :], ones1_8[:], rhs,
                             start=first, stop=last)


def _emit_ctx(nc, pools, xw, W3, tag):
    """xw/den -> xq8 -> diagonal-head G entries -> ctx [128, NF] f32 (SBUF)."""
    psG, ework = pools["psG"], pools["ework"]
    xw3 = xw[:].rearrange("p (a b) -> p a b", a=NF + 1)

    # den column holds 64*den (ones tile = 64), so inv = (1/64)/den and the
    # x64-scaled W chunks (wq8s/wk8s) cancel it in GT.
    inv = ework.tile([P, 12], F32, tag=f"inv{tag}")
    nc.vector.reciprocal(inv[:], xw[:, NF * 12:NF * 12 + 12])

    xq8 = ework.tile([P, NF * 12], F8, tag=f"xq{tag}")
    xq3 = xq8[:].rearrange("p (a b) -> p a b", a=NF)
    nc.vector.tensor_tensor(xq3, xw3[:, 0:NF, :],
                            inv[:, None, :].broadcast_to((P, NF, 12)),
                            mybir.AluOpType.mult)

    # only diagonal head pairs of G are needed: block m uses heads 2m, 2m+1
    gt = psG.tile([P, 2 * NF], F32, tag="g")
    for m in range(NF):
        for j in range(NF):
            nc.tensor.matmul(gt[:, 2 * m:2 * (m + 1)],
                             W3[:, j, P * m:P * (m + 1)],
                             xq3[:, j, 2 * m:2 * (m + 1)],
                             start=(j == 0), stop=(j == NF - 1))
    gt3 = gt[:].rearrange("p (a b) -> p a b", a=NF)
    return gt3


def build_program(masked=False):
    nc = bacc.Bacc(trn_type="TRN2", target_bir_lowering=False)

    x8t_d = nc.dram_tensor("x8t", [P, NF * S], F8, kind="ExternalInput")
    xr8t_d = nc.dram_tensor("xr8t", [P, NF * S], F8, kind="ExternalInput")
    xs8_d = nc.dram_tensor("xs8", [P, NS * F], F8, kind="ExternalInput")
    aq8_d = nc.dram_tensor("aq8", [P, NF * 12], F8, kind="ExternalInput")
    wq8_d = nc.dram_tensor("wq8", [P, NF * F], F8, kind="ExternalInput")
    wk8_d = nc.dram_tensor("wk8", [P, NF * F], F8, kind="ExternalInput")
    wkt8_d = nc.dram_tensor("wkt8", [P, NF * F], F8, kind="ExternalInput")
    wqt8_d = nc.dram_tensor("wqt8", [P, NF * F], F8, kind="ExternalInput")
    wqr8_d = nc.dram_tensor("wqr8", [P, NF * F], F8, kind="ExternalInput")
    wka_d = nc.dram_tensor("wka", [P, NF * 12], F32, kind="ExternalInput")
    wobd_d = nc.dram_tensor("wobd", [P, P], F32, kind="ExternalInput")
    ones8_d = nc.dram_tensor("ones8", [P, 2 * P], F8, kind="ExternalInput")
    lm_d = nc.dram_tensor("lm", [P, NS], F32, kind="ExternalInput")
    out_d = nc.dram_tensor("out", [S, F], F16, kind="ExternalOutput")

    with tile.TileContext(nc) as tc:
        with ExitStack() as ctx:
            cpool = ctx.enter_context(tc.tile_pool(name="const", bufs=1))
            ework = ctx.enter_context(tc.tile_pool(name="ework", bufs=1))
            obuf = ctx.enter_context(tc.tile_pool(name="obuf", bufs=3))
            psW = ctx.enter_context(tc.tile_pool(name="psW", bufs=2, space="PSUM"))

            # ---- loads, in consumption order; small tensors first
            aq8 = cpool.tile([P, NF * 12], F8, tag="aq8")
            nc.sync.dma_start(aq8[:], aq8_d[:])
            ones128_8 = cpool.tile([P, 2 * P], F8, tag="ones8")
            nc.sync.dma_start(ones128_8[:], ones8_d[:])
            lm_sb = cpool.tile([P, NS], F32, tag="lm")
            if masked:
                nc.sync.dma_start(lm_sb[:], lm_d[:])
            wka = cpool.tile([P, NF * 12], F32, tag="wka")
            nc.sync.dma_start(wka[:], wka_d[:])
            wobd = cpool.tile([P, P], F32, tag="wobd")
            nc.sync.dma_start(wobd[:], wobd_d[:])
            x8t = cpool.tile([P, NF * S], F8, tag="x8t")
            x8t3 = x8t[:].rearrange("p (a b) -> p a b", a=NF)
            x8t_d3 = x8t_d[:].rearrange("p (a b) -> p a b", a=NF)
            xs8_tiles = []
            xs8 = []
            for g in range(NG):
                t = cpool.tile([P, GS * F], F8, tag=f"xs8_{g}")
                xs8_tiles.append(t)
                xs8.append(t[:].rearrange("p (a b) -> p a b", a=GS))
            # interleave: xT half 1, xs groups 0-3, xT half 2, xs groups 4-7
            nc.sync.dma_start(x8t3[:, :, 0:S // 2], x8t_d3[:, :, 0:S // 2])
            for g in range(NG // 2):
                nc.sync.dma_start(xs8_tiles[g][:],
                                  xs8_d[:, GS * F * g:GS * F * (g + 1)])
            nc.sync.dma_start(x8t3[:, :, S // 2:S], x8t_d3[:, :, S // 2:S])
            for g in range(NG // 2, NG):
                nc.sync.dma_start(xs8_tiles[g][:],
                                  xs8_d[:, GS * F * g:GS * F * (g + 1)])

            wq8 = cpool.tile([P, NF * F], F8, tag="wq8")
            nc.sync.dma_start(wq8[:], wq8_d[:])
            wkt8 = cpool.tile([P, NF * F], F8, tag="wkt8")
            nc.sync.dma_start(wkt8[:], wkt8_d[:])
            wk8 = cpool.tile([P, NF * F], F8, tag="wk8")
            nc.sync.dma_start(wk8[:], wk8_d[:])
            wqt8 = cpool.tile([P, NF * F], F8, tag="wqt8")
            nc.sync.dma_start(wqt8[:], wqt8_d[:])
            wqr8 = cpool.tile([P, NF * F], F8, tag="wqr8")
            nc.sync.dma_start(wqr8[:], wqr8_d[:])
            xr8t = cpool.tile([P, NF * S], F8, tag="xr8t")
            xr8t3 = xr8t[:].rearrange("p (a b) -> p a b", a=NF)
            xr8t_d3 = xr8t_d[:].rearrange("p (a b) -> p a b", a=NF)
            for q in range(4):
                lo, hi = S // 4 * q, S // 4 * (q + 1)
                nc.sync.dma_start(xr8t3[:, :, lo:hi], xr8t_d3[:, :, lo:hi])

            wq3 = wq8[:].rearrange("p (a b) -> p a b", a=NF)
            wk3 = wk8[:].rearrange("p (a b) -> p a b", a=NF)
            wkt3 = wkt8[:].rearrange("p (a b) -> p a b", a=NF)
            wqt3 = wqt8[:].rearrange("p (a b) -> p a b", a=NF)
            wqr3 = wqr8[:].rearrange("p (a b) -> p a b", a=NF)
            wka3 = wka[:].rearrange("p (a b) -> p a b", a=NF)
            aq3 = aq8[:].rearrange("p (a b) -> p a b", a=NF)
            cst = {"x8t3": x8t3, "xs8": xs8, "lm_sb": lm_sb,
                   "ones1_8": ones128_8[:, 0:P]}

            mr8 = ework.tile([P, NF * F], F8, tag="mr8")
            m8_3 = wq3
            mr8_3 = mr8[:].rearrange("p (a b) -> p a b", a=NF)

            with ExitStack() as pre:
                psS = pre.enter_context(tc.tile_pool(name="psS", bufs=2,
                                                     space="PSUM"))
                psXW = pre.enter_context(tc.tile_pool(name="psXW", bufs=1,
                                                      space="PSUM"))
                psG = pre.enter_context(tc.tile_pool(name="psG", bufs=1,
                                                     space="PSUM"))
                pools = {"psS": psS, "psXW": psXW, "psG": psG, "ework": ework}

                # ---- pass 1: query pooling + q_ctx (split by xT halves)
                e_q = []
                xw_q = psXW.tile([P, (NF + 1) * 12], F32, tag="xw")
                _emit_scores(nc, pools, cst, aq3, masked, 0, e_q)
                _emit_xw(nc, pools, cst, e_q, range(NG // 2), xw_q, 0, NS - 1)
                _emit_scores(nc, pools, cst, aq3, masked, 1, e_q)
                _emit_xw(nc, pools, cst, e_q, range(NG // 2, NG), xw_q,
                         0, NS - 1)
                gtq3 = _emit_ctx(nc, pools, xw_q, wq3, "q")

                # ---- A_k = Wk @ (q_ctx * Wka): gate straight from PSUM gt
                g8 = ework.tile([P, NF * 12], F8, tag="g8")
                g3 = g8[:].rearrange("p (a b) -> p a b", a=NF)
                nc.vector.tensor_tensor(
                    g3[0:64], wka3[0:64],
                    gtq3[0:64, :, 0:1].broadcast_to((64, NF, 12)),
                    mybir.AluOpType.mult)
                nc.vector.tensor_tensor(
                    g3[64:P], wka3[64:P],
                    gtq3[64:P, :, 1:2].broadcast_to((64, NF, 12)),
                    mybir.AluOpType.mult)
                qctx = ework.tile([P, NF], F32, tag="qctx")
                nc.vector.tensor_copy(qctx[0:64, :], gtq3[0:64, :, 0])
                nc.vector.tensor_copy(qctx[64:P, :], gtq3[64:P, :, 1])
                ak_ps = psG.tile([P, NF * 12], F32, tag="g")
                for ft in range(NF):
                    for fc in range(NF):
                        nc.tensor.matmul(ak_ps[:, 12 * ft:12 * (ft + 1)],
                                         wkt3[:, fc, P * ft:P * (ft + 1)],
                                         g3[:, fc, :],
                                         start=(fc == 0), stop=(fc == NF - 1))
                ak8 = ework.tile([P, NF * 12], F8, tag="ak8")
                nc.scalar.copy(ak8[:], ak_ps[:])
                ak3 = ak8[:].rearrange("p (a b) -> p a b", a=NF)

                # ---- pass 2: key pooling + k_ctx
                e_k = []
                xw_k = psXW.tile([P, (NF + 1) * 12], F32, tag="xw")
                _emit_scores(nc, pools, cst, ak3, masked, 0, e_k)
                _emit_xw(nc, pools, cst, e_k, range(NG // 2), xw_k, 0, NS - 1)
                _emit_scores(nc, pools, cst, ak3, masked, 1, e_k)
                _emit_xw(nc, pools, cst, e_k, range(NG // 2, NG), xw_k,
                         0, NS - 1)
                gtk3 = _emit_ctx(nc, pools, xw_k, wk3, "k")
                kctx = ework.tile([P, NF], F32, tag="kctx")
                nc.vector.tensor_tensor(kctx[0:64, :], qctx[0:64, :],
                                        gtk3[0:64, :, 0],
                                        mybir.AluOpType.mult)
                nc.vector.tensor_tensor(kctx[64:P, :], qctx[64:P, :],
                                        gtk3[64:P, :, 1],
                                        mybir.AluOpType.mult)

                # ---- M = Wq @ (blockdiag(kctx_h * Wo) + I), scaled by MS
                # r_all[:, j, :] = wobd (block-diag stacked Wo, x64) row-scaled
                # by kctx[:, j]; the +I (i.e. + Wq) lands via MS*I128 matmuls.
                r_all = ework.tile([P, NF * P], F16, tag="r_all")
                r3 = r_all[:].rearrange("p (a b) -> p a b", a=NF)
                nc.vector.tensor_tensor(
                    r3, wobd[:, None, :].broadcast_to((P, NF, P)),
                    kctx[:, :, None].broadcast_to((P, NF, P)),
                    mybir.AluOpType.mult)

                for ft in range(NF):
                    mc = psW.tile([P, F], F32, tag="wide")
                    for j in range(NF):
                        nc.tensor.matmul(mc[:, P * j:P * (j + 1)],
                                         wqt3[:, j, P * ft:P * (ft + 1)],
                                         r3[:, j, :], start=True, stop=True)
                    nc.vector.tensor_tensor(mr8_3[:, ft, :], mc[:],
                                            wqr3[:, ft, :],
                                            mybir.AluOpType.add)

            # ---- pass 3: out = (x8 + xr8) @ M8 + x8 @ Mr8, fp8 DoubleRow,
            # pair-major accumulation; two pools alternate -> 4 chunks in flight
            psT = ctx.enter_context(tc.tile_pool(name="psT", bufs=2,
                                                 space="PSUM"))
            for i in range(NS):
                ps = (psW if i % 2 == 0 else psT).tile([P, F], F32, tag="wide")
                n = 0
                for t in range(NF // 2):
                    for lhs3, rhs3 in ((x8t3, m8_3), (x8t3, mr8_3),
                                      (xr8t3, m8_3)):
                        for lo, hi in ((0, 512), (512, F)):
                            nc.tensor.matmul(
                                ps[:, lo:hi],
                                lhs3[:, 2 * t:2 * t + 2, P * i:P * (i + 1)],
                                rhs3[:, 2 * t:2 * t + 2, lo:hi],
                                start=(n == 0), stop=(n == 16),
                                perf_mode=DR)
                        n += 2
                ow = obuf.tile([P, F], F16, tag="ow")
                nc.scalar.mul(ow[:], ps[:], 1.0 / MS)
                nc.sync.dma_start(out_d[P * i:P * (i + 1), :], ow[:])

    nc.compile()
    return nc


def _get_program(masked=False):
    key = ("m" if masked else "u")
    if key not in _prog_cache:
        _prog_cache[key] = build_program(masked)
    return _prog_cache[key]


def _chunk_rows(a, np_dtype):
    """[R*128, C] -> [128, R*C] with chunk r of rows at cols [r*C:(r+1)*C]."""
    R = a.shape[0] // P
    return np.ascontiguousarray(
        a.reshape(R, P, a.shape[1]).transpose(1, 0, 2).reshape(P, -1)
    ).astype(np_dtype)


def _prep_weights(Wq, Wk, Wqa, Wka, Wo):
    Aq = (Wq @ Wqa).astype(np.float32)
    wobd = np.zeros((P, P), np.float32)
    wobd[0:64, 0:64] = MS * Wo
    wobd[64:P, 64:P] = MS * Wo
    return {
        "aq8": _chunk_rows(Aq, NP8),
        "wq8": _chunk_rows(MS * Wq, NP8),
        "wk8": _chunk_rows(MS * Wk, NP8),
        "wkt8": _chunk_rows(np.ascontiguousarray(Wk.T), NP8),
        "wqt8": _chunk_rows(np.ascontiguousarray(Wq.T), NP8),
        "wqr8": _chunk_rows(
            MS * Wq - (MS * Wq).astype(NP8).astype(np.float32), NP8),
        "wka": _chunk_rows(Wka, np.float32),
        "wobd": wobd,
        "ones8": np.full((P, 2 * P), MS, NP8),
    }


def _prep_core_inputs(xb, maskb, w, masked):
    x8 = xb.astype(NP8)
    xr8 = (xb - x8.astype(np.float32)).astype(NP8)
    d = {
        "x8t": _chunk_rows(np.ascontiguousarray(x8.astype(np.float32).T), NP8),
        "xr8t": _chunk_rows(np.ascontiguousarray(xr8.astype(np.float32).T), NP8),
        "xs8": _chunk_rows(x8.astype(np.float32), NP8),
        "lm": np.zeros((P, NS), np.float32),
        **w,
    }
    if masked:
        lm = np.where(maskb > 0, 0.0, -60000.0).astype(np.float32) * ESC
        d["lm"] = np.ascontiguousarray(lm.reshape(NS, P).T)
    return d


def run(x, attn_mask, Wq, Wk, Wqa, Wka, Wo, trace=False):
    from concourse.bass_utils import run_bass_kernel_spmd

    masked = not bool(np.all(attn_mask == 1.0))
    nc = _get_program(masked)
    w = _prep_weights(Wq, Wk, Wqa, Wka, Wo)
    in_maps = [_prep_core_inputs(np.asarray(x[b]), np.asarray(attn_mask[b]),
                                 w, masked)
               for b in range(N_CORES)]
    res = run_bass_kernel_spmd(nc, in_maps, list(range(N_CORES)), trace=trace)
    out = np.stack([res.results[b]["out"].astype(np.float32)
                    for b in range(N_CORES)])
    return out, res


def kernel(x, attn_mask, Wq, Wk, Wqa, Wka, Wo):
    out, _ = run(np.asarray(x, dtype=np.float32),
                 np.asarray(attn_mask, dtype=np.float32),
                 np.asarray(Wq, dtype=np.float32),
                 np.asarray(Wk, dtype=np.float32),
                 np.asarray(Wqa, dtype=np.float32),
                 np.asarray(Wka, dtype=np.float32),
                 np.asarray(Wo, dtype=np.float32))
    return out
